# revision 20
# baseline (speedup 1.0000x reference)
"""GroupQueryAttention on 8 TRN2 NeuronCores.

Strategy: tensor-parallel over heads. H=32 query heads, KV=8 kv heads,
group size G=4 -> each core owns exactly 1 kv head and its 4 query heads.
Per core:
  - QKV projections from a replicated (pre-transposed, channels-major) input
  - RoPE on Q/K (rotate-half, done on DVE across partition halves)
  - attention with scores computed TRANSPOSED ([keys, q] layout) so the
    exp(scores) tiles feed the V-matmul directly as the moving operand;
    softmax normalization is deferred: O = V.E, then ctx = O * (1/colsum(E))
  - partial output ctx @ Wo_shard  (row-shard of Wo)
Host sums the 8 partial outputs (the "all-reduce" of the row-parallel Wo).

Perf structure (v3):
  - softmax normalization: colsum matmul (ones) -> DVE reciprocal_approx_fast
    -> rank-1 broadcast matmul -> ACT copy -> DVE scale. Short chain, no
    activation-table thrash (only Exp is ever used on ACT), no long DVE
    iterative divides
  - causal diagonal tiles trimmed: score/PV matmuls only cover the
    not-fully-masked query columns; a single 128x128 lower-tri mask
  - per-head projection emission (K, V, then Q_h right before head h's
    attention) so startup overlaps DMA, and out-projection of quarter q-1
    interleaves into quarter q's attention as PE filler
  - PSUM->SBUF drains split between ACT and DVE; output rows coalesced
    into [128, 2048] tiles before DMA
"""

import sys

sys.path.insert(0, "/opt/trn_rl_repo")

from contextlib import ExitStack

import numpy as np
import ml_dtypes

import concourse.bass as bass
import concourse.bacc as bacc
import concourse.tile as tile
from concourse import mybir
from concourse import bass_isa
from concourse.bass_utils import run_bass_kernel_spmd

BF16 = ml_dtypes.bfloat16

S = 2048          # sequence length
DIN = 4096        # model dim
H, KV, DH = 32, 8, 128
G = H // KV       # 4 query heads per kv head
NCORES = 8
HPC = H // NCORES     # 4 query heads per core
DPC = HPC * DH        # 512 = per-core q-projection width

NQ = 4            # s-quarters (chunks of 512 queries)
QC = S // NQ      # 512
KT = 128          # key tile (partition dim of transposed scores)
NKT = S // KT     # 16 key tiles
NK = DIN // 128   # 32 contraction tiles for projections
NXC = 8           # x chunks per quarter (k-groups of CW)
CW = NK // NXC    # 4 k-tiles per x chunk
SCALE = 1.0 / float(np.sqrt(DH))
EXP_BIAS = -10.0  # constant shift inside exp; cancels in normalization


def build_nc():
    """Build the per-core Bass program (same program on all 8 cores; the
    per-core weight shards arrive via in_maps)."""
    nc = bacc.Bacc()
    dt = mybir.dt

    # ---- DRAM parameters (host-prepared layouts; all DMA-contiguous) ----
    # x[p, sq, k, sc] = x_orig[512*sq + sc, 128*k + p]   (channels-major)
    x = nc.declare_dram_parameter("x", [128, NQ, NK, QC], dt.bfloat16, isOutput=False)
    # wq[p, k, m, d] = Wq_shard[128*k + p, 128*m + d]
    wq = nc.declare_dram_parameter("wq", [128, NK, HPC, DH], dt.bfloat16, isOutput=False)
    # wk[p, k, d] = Wk_shard[128*k + p, d]
    wk = nc.declare_dram_parameter("wk", [128, NK, DH], dt.bfloat16, isOutput=False)
    wv = nc.declare_dram_parameter("wv", [128, NK, DH], dt.bfloat16, isOutput=False)
    # wo[p, h, n] = Wo_shard[128*h + p, n]
    wo = nc.declare_dram_parameter("wo", [128, HPC, DIN], dt.bfloat16, isOutput=False)
    # cosT[d, s] = cos[s, d]; sinm[d, s] = -sin[s, d] for d<64 else +sin[s, d]
    cosT = nc.declare_dram_parameter("cosT", [DH, S], dt.float32, isOutput=False)
    sinm = nc.declare_dram_parameter("sinm", [DH, S], dt.float32, isOutput=False)
    # tri[p, c] = 1.0 if p <= c else 0.0  (128x128 causal triangle)
    tri = nc.declare_dram_parameter("tri", [128, 128], dt.bfloat16, isOutput=False)
    ident = nc.declare_dram_parameter("ident", [128, 128], dt.bfloat16, isOutput=False)
    ones_col = nc.declare_dram_parameter("ones_col", [128, 1], dt.bfloat16, isOutput=False)
    out = nc.declare_dram_parameter("out", [S, DIN], dt.bfloat16, isOutput=True)

    with tile.TileContext(nc) as tc, ExitStack() as ctx:
        singles = ctx.enter_context(tc.tile_pool(name="singles", bufs=1))
        wqp = ctx.enter_context(tc.tile_pool(name="wqp", bufs=1))
        wop = ctx.enter_context(tc.tile_pool(name="wop", bufs=1))
        xp = ctx.enter_context(tc.tile_pool(name="xp", bufs=1))
        qkv = ctx.enter_context(tc.tile_pool(name="qkv", bufs=1))
        epool = ctx.enter_context(tc.tile_pool(name="epool", bufs=4))
        spool = ctx.enter_context(tc.tile_pool(name="spool", bufs=2))
        npool = ctx.enter_context(tc.tile_pool(name="npool", bufs=2))
        tpool = ctx.enter_context(tc.tile_pool(name="tpool", bufs=2))
        obp = ctx.enter_context(tc.tile_pool(name="obp", bufs=3))
        ps_proj = ctx.enter_context(tc.tile_pool(name="ps_proj", bufs=2, space="PSUM"))
        ps_po = ctx.enter_context(tc.tile_pool(name="ps_po", bufs=2, space="PSUM"))
        ps_sc = ctx.enter_context(tc.tile_pool(name="ps_sc", bufs=2, space="PSUM"))
        ps_out = ctx.enter_context(tc.tile_pool(name="ps_out", bufs=2, space="PSUM"))

        # ---- constants / weights resident in SBUF ----
        # DMA emission order == consumption order so the PE never stalls at
        # kernel start: wk + x chunks feed the K-projection, wv + wq_h0
        # right after, cos/sin for RoPE before quarter-0 attention; the
        # remaining q-head weights and wo trail in.
        w_k = singles.tile([128, NK, DH], dt.bfloat16, tag="wk")
        w_v = singles.tile([128, NK, DH], dt.bfloat16, tag="wv")
        w_q = [wqp.tile([128, NK, DH], dt.bfloat16, tag=f"wq{h}", name=f"wq{h}")
               for h in range(HPC)]
        c_cos = singles.tile([DH, S], dt.float32, tag="cos")
        c_sin = singles.tile([DH, S], dt.float32, tag="sin")

        nc.sync.dma_start(out=w_k[:, 0:8], in_=wk[:, 0:8])
        xcs0 = [xp.tile([128, NK // NXC, QC], dt.bfloat16, tag=f"xc{g}",
                        name=f"xc{g}_0") for g in range(NXC)]
        nc.sync.dma_start(out=xcs0[0], in_=x[:, 0, 0:CW])
        nc.sync.dma_start(out=w_v, in_=wv[:])
        nc.sync.dma_start(out=xcs0[1], in_=x[:, 0, CW:2 * CW])
        nc.sync.dma_start(out=w_q[0], in_=wq[:, :, 0])
        fillers = [("wk1", None), ("cos", None), ("wk2", None), ("sin", None),
                   ("wk3", None)]
        fi = 0
        for g in range(2, NXC):
            nc.sync.dma_start(out=xcs0[g], in_=x[:, 0, g * CW:(g + 1) * CW])
            if fi < len(fillers):
                nm = fillers[fi][0]
                if nm.startswith("wk"):
                    gg = int(nm[2])
                    nc.sync.dma_start(out=w_k[:, 8 * gg:8 * (gg + 1)],
                                      in_=wk[:, 8 * gg:8 * (gg + 1)])
                elif nm == "cos":
                    nc.sync.dma_start(out=c_cos, in_=cosT[:])
                elif nm == "sin":
                    nc.sync.dma_start(out=c_sin, in_=sinm[:])
                fi += 1

        c_tri = singles.tile([128, 128], dt.bfloat16, tag="tri")
        nc.sync.dma_start(out=c_tri, in_=tri[:])
        c_id = singles.tile([128, 128], dt.bfloat16, tag="ident")
        nc.sync.dma_start(out=c_id, in_=ident[:])
        c_oc = singles.tile([128, 1], dt.bfloat16, tag="ones_col")
        nc.sync.dma_start(out=c_oc, in_=ones_col[:])

        for h in range(1, HPC):
            nc.sync.dma_start(out=w_q[h], in_=wq[:, :, h])

        w_o = wop.tile([128, HPC, DIN], dt.bfloat16, tag="wo")
        for h in range(HPC):
            nc.sync.dma_start(out=w_o[:, h], in_=wo[:, h])

        c_bias = singles.tile([128, 1], dt.float32, tag="ebias")
        nc.vector.memset(c_bias, EXP_BIAS)

        # ---- long-lived activations ----
        qt = [qkv.tile([DH, S], dt.bfloat16, tag=f"qt{h}", name=f"qt{h}")
              for h in range(HPC)]
        kt = qkv.tile([DH, S], dt.bfloat16, tag="kt")
        vn = qkv.tile([128, NKT, DH], dt.bfloat16, tag="vn")   # V natural tiles
        ctxT = [qkv.tile([DH, S], dt.bfloat16, tag=f"ctx{h}", name=f"ctx{h}")
                for h in range(HPC)]

        def rope_from_psum(ps, dst_slice, s0):
            """dst = ps*cos + rot_half(ps)*sinm over s-columns [s0, s0+QC)."""
            t1 = tpool.tile([DH, QC], dt.float32, tag="t1", name="t1")
            nc.vector.tensor_mul(t1, ps, c_cos[:, s0:s0 + QC])
            t2 = tpool.tile([DH, QC], dt.float32, tag="t2", name="t2")
            nc.vector.tensor_mul(t2[0:64, :], ps[64:128, :], c_sin[0:64, s0:s0 + QC])
            nc.vector.tensor_mul(t2[64:128, :], ps[0:64, :], c_sin[64:128, s0:s0 + QC])
            nc.vector.tensor_add(dst_slice, t1, t2)

        def emit_kv(sq, xcs):
            """K and V projections + RoPE(K) + V transpose for quarter sq."""
            s0 = sq * QC
            psk = ps_proj.tile([DH, QC], dt.float32, tag="acc", name="psk")
            for k in range(NK):
                nc.tensor.matmul(psk, lhsT=w_k[:, k], rhs=xcs[k // CW][:, k % CW],
                                 start=(k == 0), stop=(k == NK - 1))
            rope_from_psum(psk, kt[:, s0:s0 + QC], s0)

            psv = ps_proj.tile([DH, QC], dt.float32, tag="acc", name="psv")
            for k in range(NK):
                nc.tensor.matmul(psv, lhsT=w_v[:, k], rhs=xcs[k // CW][:, k % CW],
                                 start=(k == 0), stop=(k == NK - 1))
            vtmp = tpool.tile([DH, QC], dt.bfloat16, tag="vtmp", name="vtmp")
            nc.scalar.copy(vtmp, psv)
            for i in range(QC // 128):
                pvt = ps_sc.tile([128, 128], dt.bfloat16, tag="sc", name="pvt")
                nc.tensor.transpose(pvt, vtmp[:, i * 128:(i + 1) * 128], c_id)
                nc.vector.tensor_copy(vn[:, sq * 4 + i], pvt)

        def emit_q(sq, h, xcs):
            """Q projection + RoPE for head h, quarter sq.

            The x-chunk read order is rotated per head so that the last Q
            stream (h3) reads chunk 0 first -> next quarter's chunk-0
            prefetch DMA (WAR on these reads) can start that much sooner,
            staggered in the same order the next quarter consumes them."""
            s0 = sq * QC
            psq = ps_proj.tile([DH, QC], dt.float32, tag="acc", name="psq")
            korder = [CW * ((2 * (h + 1) + g) % NXC) + kk
                      for g in range(NXC) for kk in range(CW)]
            for i, k in enumerate(korder):
                nc.tensor.matmul(psq, lhsT=w_q[h][:, k],
                                 rhs=xcs[k // CW][:, k % CW],
                                 start=(i == 0), stop=(i == NK - 1))
            rope_from_psum(psq, qt[h][:, s0:s0 + QC], s0)

        def emit_attn_head(sq, h):
            """Causal attention for head h over quarter sq's queries.

            Scores are [key-tile, q] transposed; diagonal key-tiles are
            trimmed to the columns that aren't fully masked, and the
            128-wide triangle on the diagonal gets the 0/1 mask."""
            s0 = sq * QC
            njt = 4 * (sq + 1)
            sacc = spool.tile([128, QC], dt.bfloat16, tag="sacc", name="sacc")
            po = ps_po.tile([DH, QC], dt.float32, tag="po", name="po")
            for jt in range(njt):
                r = jt - (njt - 4)          # >=0 -> diagonal tile index
                c0 = 128 * r if r > 0 else 0
                psc = ps_sc.tile([128, QC], dt.float32, tag="sc", name="psc")
                nc.tensor.matmul(psc[:, c0:QC], lhsT=kt[:, jt * KT:(jt + 1) * KT],
                                 rhs=qt[h][:, s0 + c0:s0 + QC],
                                 start=True, stop=True)
                e = epool.tile([128, QC], dt.bfloat16, tag="e", name="e")
                nc.scalar.activation(out=e[:, c0:QC], in_=psc[:, c0:QC],
                                     func=mybir.ActivationFunctionType.Exp,
                                     bias=c_bias, scale=SCALE)
                if r >= 0:
                    nc.vector.tensor_mul(e[:, c0:c0 + 128], e[:, c0:c0 + 128],
                                         c_tri)
                if jt == 0:
                    nc.vector.tensor_copy(sacc, e[:, 0:QC])
                else:
                    nc.vector.tensor_add(sacc[:, c0:QC], sacc[:, c0:QC],
                                         e[:, c0:QC])
                nc.tensor.matmul(po[:, c0:QC], lhsT=vn[:, jt], rhs=e[:, c0:QC],
                                 start=(jt == 0), stop=(jt == njt - 1))
            # normalization: ctx = O * (1/colsum(E)). colsum via a ones
            # matmul, fast approximate reciprocal on DVE (single custom op,
            # ~51 ULP), partition-broadcast on the otherwise-idle gpsimd,
            # DVE scale. Nothing expensive on any engine, no PE wait.
            pcs = ps_out.tile([1, QC], dt.float32, tag="pso", name="pcs")
            nc.tensor.matmul(pcs, lhsT=c_oc, rhs=sacc, start=True, stop=True)
            rec = npool.tile([1, QC], dt.float32, tag="rec", name="rec")
            nc.vector.reciprocal_approx_fast(out=rec, in_=pcs)
            rcs = npool.tile([128, QC], dt.float32, tag="nrm", name="rcs")
            nc.gpsimd.partition_broadcast(rcs, rec)
            nc.vector.tensor_mul(ctxT[h][:, s0:s0 + QC], po, rcs)

        def emit_outproj_block(st, alt_pool=None):
            """out[st*128:(st+1)*128, :] = sum_h ctxT[h][:, st-block].T @ Wo[h]."""
            for quad in range(4):
                ob = obp.tile([128, DIN // 4], dt.bfloat16, tag="ob", name="ob")
                for j in range(2):
                    oc = quad * 2 + j
                    if alt_pool is not None and oc % 2:
                        pso = alt_pool.tile([128, 512], dt.float32, tag="acc",
                                            name="pso_alt")
                    else:
                        pso = ps_out.tile([128, 512], dt.float32, tag="pso",
                                          name="pso")
                    for h in range(HPC):
                        nc.tensor.matmul(pso,
                                         lhsT=ctxT[h][:, st * 128:(st + 1) * 128],
                                         rhs=w_o[:, h, oc * 512:(oc + 1) * 512],
                                         start=(h == 0), stop=(h == HPC - 1))
                    dst = ob[:, j * 512:(j + 1) * 512]
                    if oc % 2 == 0:
                        nc.scalar.copy(dst, pso)
                    else:
                        nc.vector.tensor_copy(dst, pso)
                nc.sync.dma_start(
                    out=out[st * 128:(st + 1) * 128,
                            quad * (DIN // 4):(quad + 1) * (DIN // 4)],
                    in_=ob)

        # ---- main pipeline ----
        # Per quarter: K/V projections, then per head [Q_h, attention_h,
        # out-proj block of the previous quarter]. The out-proj matmuls are
        # independent PE filler while attention waits on exp results; the
        # x chunks for the next quarter prefetch once the last Q reads them.
        def prefetch_x(sq1, gs):
            tiles = []
            for g in gs:
                xc = xp.tile([128, NK // NXC, QC], dt.bfloat16,
                             tag=f"xc{g}", name=f"xc{g}_{sq1}")
                nc.sync.dma_start(out=xc, in_=x[:, sq1, g * CW:(g + 1) * CW])
                tiles.append(xc)
            return tiles

        xcs = xcs0
        for sq in range(NQ):
            emit_kv(sq, xcs)
            for h in range(HPC):
                emit_q(sq, h, xcs)
                if h == HPC - 1 and sq + 1 < NQ:
                    # stagger the prefetch issue so the first chunks get
                    # full DMA bandwidth and land before the next K-proj
                    nxt = prefetch_x(sq + 1, range(0, 3))
                emit_attn_head(sq, h)
                # out-proj blocks of the previous quarter fill PE bubbles at
                # heads 0-2; block 4*sq at the quarter boundary (right after
                # head 3) covers the gap while the next quarter's x arrives.
                if sq >= 1 and h < HPC - 1:
                    emit_outproj_block(4 * (sq - 1) + h + 1)
            if sq + 1 < NQ:
                nxt += prefetch_x(sq + 1, range(3, 6))
            emit_outproj_block(4 * sq)
            if sq + 1 < NQ:
                nxt += prefetch_x(sq + 1, range(6, NXC))
                xcs = nxt
        for st in range(4 * (NQ - 1) + 1, 4 * NQ):
            emit_outproj_block(st, alt_pool=ps_proj)
    nc.finalize()
    return nc


def make_in_maps(input_tensor, cos, sin, Wq, Wk, Wv, Wo):
    """Host-side sharding + layout preparation. Returns list of 8 dicts."""
    x2 = np.ascontiguousarray(input_tensor.reshape(S, DIN))
    # x_host[p, sq, k, sc] = x2[512*sq+sc, 128*k+p]
    xt = x2.T.astype(BF16)                      # [DIN, S]
    x_host = np.ascontiguousarray(
        xt.reshape(NK, 128, NQ, QC).transpose(1, 2, 0, 3))

    cosT = np.ascontiguousarray(cos.T.astype(np.float32))
    sinm = np.ascontiguousarray(sin.T.astype(np.float32))
    sinm = sinm.copy()
    sinm[0:64, :] *= -1.0

    p_idx = np.arange(128)[:, None]
    c_idx = np.arange(128)[None, :]
    tri = (p_idx <= c_idx).astype(BF16)

    ident = np.eye(128, dtype=BF16)
    ones_col = np.ones((128, 1), dtype=BF16)

    common = dict(x=x_host, cosT=cosT, sinm=sinm, tri=tri, ident=ident,
                  ones_col=ones_col)

    in_maps = []
    for c in range(NCORES):
        wq_s = Wq[:, c * DPC:(c + 1) * DPC].astype(BF16)
        wq_host = np.ascontiguousarray(
            wq_s.reshape(NK, 128, HPC, DH).transpose(1, 0, 2, 3))
        wk_s = Wk[:, c * DH:(c + 1) * DH].astype(BF16)
        wk_host = np.ascontiguousarray(wk_s.reshape(NK, 128, DH).transpose(1, 0, 2))
        wv_s = Wv[:, c * DH:(c + 1) * DH].astype(BF16)
        wv_host = np.ascontiguousarray(wv_s.reshape(NK, 128, DH).transpose(1, 0, 2))
        wo_s = Wo[c * DPC:(c + 1) * DPC, :].astype(BF16)
        wo_host = np.ascontiguousarray(wo_s.reshape(HPC, 128, DIN).transpose(1, 0, 2))
        in_maps.append(dict(common, wq=wq_host, wk=wk_host, wv=wv_host, wo=wo_host))
    return in_maps


def _numpy_fallback(input_tensor, attention_mask, cos, sin, Wq, Wk, Wv, Wo):
    x = input_tensor.astype(np.float32)
    b, s, _ = x.shape
    q = (x @ Wq).reshape(b, s, H, DH).transpose(0, 2, 1, 3)
    k = (x @ Wk).reshape(b, s, KV, DH).transpose(0, 2, 1, 3)
    v = (x @ Wv).reshape(b, s, KV, DH).transpose(0, 2, 1, 3)

    def rope(t):
        t1, t2 = t[..., :64], t[..., 64:]
        rot = np.concatenate([-t2, t1], axis=-1)
        return t * cos[None, None] + rot * sin[None, None]

    q, k = rope(q), rope(k)
    k = np.repeat(k, G, axis=1)
    v = np.repeat(v, G, axis=1)
    sc = np.einsum('bhqd,bhkd->bhqk', q, k)
    sc = np.where(attention_mask, -np.inf, sc) / np.float32(np.sqrt(DH))
    sc = sc - sc.max(axis=-1, keepdims=True)
    w = np.exp(sc)
    w = w / w.sum(axis=-1, keepdims=True)
    ctx = np.einsum('bhqk,bhkd->bhqd', w, v)
    ctx = ctx.transpose(0, 2, 1, 3).reshape(b, s, H * DH)
    return (ctx @ Wo).astype(np.float32)


_NC_CACHE = {}


def kernel(input_tensor, attention_mask, cos, sin, Wq, Wk, Wv, Wo):
    mask = np.asarray(attention_mask).reshape(S, S)
    causal = np.array_equal(mask, np.triu(np.ones((S, S), bool), k=1))
    if not causal:
        return _numpy_fallback(np.asarray(input_tensor), np.asarray(attention_mask),
                               np.asarray(cos), np.asarray(sin),
                               np.asarray(Wq), np.asarray(Wk),
                               np.asarray(Wv), np.asarray(Wo))

    if "nc" not in _NC_CACHE:
        _NC_CACHE["nc"] = build_nc()
    nc = _NC_CACHE["nc"]

    in_maps = make_in_maps(np.asarray(input_tensor), np.asarray(cos),
                           np.asarray(sin), np.asarray(Wq), np.asarray(Wk),
                           np.asarray(Wv), np.asarray(Wo))
    res = run_bass_kernel_spmd(nc, in_maps, core_ids=list(range(NCORES)))
    acc = np.zeros((S, DIN), np.float32)
    for r in res.results:
        acc += np.asarray(r["out"], dtype=np.float32)
    return acc.reshape(1, S, DIN)


# revision 22
# speedup vs baseline: 1.1937x; 1.1937x over previous
"""GroupQueryAttention on 8 TRN2 NeuronCores.

Strategy: tensor-parallel over heads. H=32 query heads, KV=8 kv heads,
group size G=4 -> each core owns exactly 1 kv head and its 4 query heads.
Per core:
  - QKV projections from a replicated (pre-transposed, channels-major) input
  - RoPE on Q/K (rotate-half, done on DVE across partition halves)
  - attention with scores computed TRANSPOSED ([keys, q] layout) so the
    exp(scores) tiles feed the V-matmul directly as the moving operand;
    softmax normalization is deferred: O = V.E, then ctx = O * (1/colsum(E))
  - partial output ctx @ Wo_shard  (row-shard of Wo)
Host sums the 8 partial outputs (the "all-reduce" of the row-parallel Wo).

Perf structure (v3):
  - softmax normalization: colsum matmul (ones) -> DVE reciprocal_approx_fast
    -> rank-1 broadcast matmul -> ACT copy -> DVE scale. Short chain, no
    activation-table thrash (only Exp is ever used on ACT), no long DVE
    iterative divides
  - causal diagonal tiles trimmed: score/PV matmuls only cover the
    not-fully-masked query columns; a single 128x128 lower-tri mask
  - per-head projection emission (K, V, then Q_h right before head h's
    attention) so startup overlaps DMA, and out-projection of quarter q-1
    interleaves into quarter q's attention as PE filler
  - PSUM->SBUF drains split between ACT and DVE; output rows coalesced
    into [128, 2048] tiles before DMA
"""

import sys

sys.path.insert(0, "/opt/trn_rl_repo")

from contextlib import ExitStack

import numpy as np
import ml_dtypes

import concourse.bass as bass
import concourse.bacc as bacc
import concourse.tile as tile
from concourse import mybir
from concourse import bass_isa
from concourse.bass_utils import run_bass_kernel_spmd

BF16 = ml_dtypes.bfloat16

S = 2048          # sequence length
DIN = 4096        # model dim
H, KV, DH = 32, 8, 128
G = H // KV       # 4 query heads per kv head
NCORES = 8
HPC = H // NCORES     # 4 query heads per core
DPC = HPC * DH        # 512 = per-core q-projection width

NQ = 4            # s-quarters (chunks of 512 queries)
QC = S // NQ      # 512
KT = 128          # key tile (partition dim of transposed scores)
NKT = S // KT     # 16 key tiles
NK = DIN // 128   # 32 contraction tiles for projections
NXC = 8           # x chunks per quarter (k-groups of CW)
CW = NK // NXC    # 4 k-tiles per x chunk
SCALE = 1.0 / float(np.sqrt(DH))
EXP_BIAS = -10.0  # constant shift inside exp; cancels in normalization


def build_nc():
    """Build the per-core Bass program (same program on all 8 cores; the
    per-core weight shards arrive via in_maps)."""
    nc = bacc.Bacc()
    dt = mybir.dt

    # ---- DRAM parameters (host-prepared layouts; all DMA-contiguous) ----
    # x[p, sq, k, sc] = x_orig[512*sq + sc, 128*k + p]   (channels-major)
    x = nc.declare_dram_parameter("x", [128, NQ, NK, QC], dt.bfloat16, isOutput=False)
    # wq[p, k, m, d] = Wq_shard[128*k + p, 128*m + d]
    wq = nc.declare_dram_parameter("wq", [128, NK, HPC, DH], dt.bfloat16, isOutput=False)
    # wk[p, k, d] = Wk_shard[128*k + p, d]
    wk = nc.declare_dram_parameter("wk", [128, NK, DH], dt.bfloat16, isOutput=False)
    wv = nc.declare_dram_parameter("wv", [128, NK, DH], dt.bfloat16, isOutput=False)
    # wo[p, h, n] = Wo_shard[128*h + p, n]
    wo = nc.declare_dram_parameter("wo", [128, HPC, DIN], dt.bfloat16, isOutput=False)
    # cosT[d, s] = cos[s, d]; sinm[d, s] = -sin[s, d] for d<64 else +sin[s, d]
    cosT = nc.declare_dram_parameter("cosT", [DH, S], dt.float32, isOutput=False)
    sinm = nc.declare_dram_parameter("sinm", [DH, S], dt.float32, isOutput=False)
    # tri[p, c] = 1.0 if p <= c else 0.0  (128x128 causal triangle)
    tri = nc.declare_dram_parameter("tri", [128, 128], dt.bfloat16, isOutput=False)
    ident = nc.declare_dram_parameter("ident", [128, 128], dt.bfloat16, isOutput=False)
    ones_col = nc.declare_dram_parameter("ones_col", [128, 1], dt.bfloat16, isOutput=False)
    out = nc.declare_dram_parameter("out", [S, DIN], dt.bfloat16, isOutput=True)

    with tile.TileContext(nc) as tc, ExitStack() as ctx:
        singles = ctx.enter_context(tc.tile_pool(name="singles", bufs=1))
        wqp = ctx.enter_context(tc.tile_pool(name="wqp", bufs=1))
        wop = ctx.enter_context(tc.tile_pool(name="wop", bufs=1))
        xp = ctx.enter_context(tc.tile_pool(name="xp", bufs=1))
        qkv = ctx.enter_context(tc.tile_pool(name="qkv", bufs=1))
        epool = ctx.enter_context(tc.tile_pool(name="epool", bufs=4))
        spool = ctx.enter_context(tc.tile_pool(name="spool", bufs=2))
        npool = ctx.enter_context(tc.tile_pool(name="npool", bufs=2))
        tpool = ctx.enter_context(tc.tile_pool(name="tpool", bufs=2))
        obp = ctx.enter_context(tc.tile_pool(name="obp", bufs=3))
        ps_proj = ctx.enter_context(tc.tile_pool(name="ps_proj", bufs=2, space="PSUM"))
        ps_po = ctx.enter_context(tc.tile_pool(name="ps_po", bufs=2, space="PSUM"))
        ps_sc = ctx.enter_context(tc.tile_pool(name="ps_sc", bufs=2, space="PSUM"))
        ps_out = ctx.enter_context(tc.tile_pool(name="ps_out", bufs=2, space="PSUM"))

        # ---- constants / weights resident in SBUF ----
        # DMA emission order == consumption order so the PE never stalls at
        # kernel start: wk + x chunks feed the K-projection, wv + wq_h0
        # right after, cos/sin for RoPE before quarter-0 attention; the
        # remaining q-head weights and wo trail in.
        w_k = singles.tile([128, NK, DH], dt.bfloat16, tag="wk")
        w_v = singles.tile([128, NK, DH], dt.bfloat16, tag="wv")
        w_q = [wqp.tile([128, NK, DH], dt.bfloat16, tag=f"wq{h}", name=f"wq{h}")
               for h in range(HPC)]
        c_cos = singles.tile([DH, S], dt.float32, tag="cos")
        c_sin = singles.tile([DH, S], dt.float32, tag="sin")

        nc.sync.dma_start(out=w_k[:, 0:8], in_=wk[:, 0:8])
        xcs0 = [xp.tile([128, NK // NXC, QC], dt.bfloat16, tag=f"xc{g}",
                        name=f"xc{g}_0") for g in range(NXC)]
        nc.sync.dma_start(out=xcs0[0], in_=x[:, 0, 0:CW])
        nc.sync.dma_start(out=w_v, in_=wv[:])
        nc.sync.dma_start(out=xcs0[1], in_=x[:, 0, CW:2 * CW])
        nc.sync.dma_start(out=w_q[0], in_=wq[:, :, 0])
        fillers = [("wk1", None), ("cos", None), ("wk2", None), ("sin", None),
                   ("wk3", None)]
        fi = 0
        for g in range(2, NXC):
            nc.sync.dma_start(out=xcs0[g], in_=x[:, 0, g * CW:(g + 1) * CW])
            if fi < len(fillers):
                nm = fillers[fi][0]
                if nm.startswith("wk"):
                    gg = int(nm[2])
                    nc.sync.dma_start(out=w_k[:, 8 * gg:8 * (gg + 1)],
                                      in_=wk[:, 8 * gg:8 * (gg + 1)])
                elif nm == "cos":
                    nc.sync.dma_start(out=c_cos, in_=cosT[:])
                elif nm == "sin":
                    nc.sync.dma_start(out=c_sin, in_=sinm[:])
                fi += 1

        c_tri = singles.tile([128, 128], dt.bfloat16, tag="tri")
        nc.sync.dma_start(out=c_tri, in_=tri[:])
        c_id = singles.tile([128, 128], dt.bfloat16, tag="ident")
        nc.sync.dma_start(out=c_id, in_=ident[:])
        c_oc = singles.tile([128, 1], dt.bfloat16, tag="ones_col")
        nc.sync.dma_start(out=c_oc, in_=ones_col[:])

        for h in range(1, HPC):
            nc.sync.dma_start(out=w_q[h], in_=wq[:, :, h])

        w_o = wop.tile([128, HPC, DIN], dt.bfloat16, tag="wo")
        for h in range(HPC):
            nc.sync.dma_start(out=w_o[:, h], in_=wo[:, h])

        # ---- PE warm-up + exp-bias constant ----
        # ~50 tiny matmuls during the initial DMA wait keep the PE busy so
        # the HAM clock-gate releases (1.2 -> 2.4 GHz) before the first real
        # matmul. They accumulate zeros and finally ones^T @ (EXP_BIAS/128),
        # producing the exp bias vector -- a live chain, so nothing is DCE'd.
        w1 = singles.tile([128, 128], dt.bfloat16, tag="warm1")
        nc.vector.memset(w1, 1.0)
        wz = singles.tile([128, 1], dt.bfloat16, tag="warmz")
        nc.vector.memset(wz, 0.0)
        wb = singles.tile([128, 1], dt.bfloat16, tag="warmb")
        nc.vector.memset(wb, EXP_BIAS / 128.0)
        ps_bias = ps_sc.tile([128, 1], dt.float32, tag="sc", name="ps_bias")
        NWARM = 50
        for i in range(NWARM):
            nc.tensor.matmul(ps_bias, lhsT=w1,
                             rhs=(wz if i < NWARM - 1 else wb),
                             start=(i == 0), stop=(i == NWARM - 1))
        c_bias = singles.tile([128, 1], dt.float32, tag="ebias")
        nc.scalar.copy(c_bias, ps_bias)

        # ---- long-lived activations ----
        qt = [qkv.tile([DH, S], dt.bfloat16, tag=f"qt{h}", name=f"qt{h}")
              for h in range(HPC)]
        kt = qkv.tile([DH, S], dt.bfloat16, tag="kt")
        vn = qkv.tile([128, NKT, DH], dt.bfloat16, tag="vn")   # V natural tiles
        ctxT = [qkv.tile([DH, S], dt.bfloat16, tag=f"ctx{h}", name=f"ctx{h}")
                for h in range(HPC)]

        def rope_from_psum(ps, dst_slice, s0):
            """dst = ps*cos + rot_half(ps)*sinm over s-columns [s0, s0+QC)."""
            t1 = tpool.tile([DH, QC], dt.float32, tag="t1", name="t1")
            nc.vector.tensor_mul(t1, ps, c_cos[:, s0:s0 + QC])
            t2 = tpool.tile([DH, QC], dt.float32, tag="t2", name="t2")
            nc.vector.tensor_mul(t2[0:64, :], ps[64:128, :], c_sin[0:64, s0:s0 + QC])
            nc.vector.tensor_mul(t2[64:128, :], ps[0:64, :], c_sin[64:128, s0:s0 + QC])
            nc.vector.tensor_add(dst_slice, t1, t2)

        def emit_kv(sq, xcs):
            """K and V projections + RoPE(K) + V transpose for quarter sq."""
            s0 = sq * QC
            psk = ps_proj.tile([DH, QC], dt.float32, tag="acc", name="psk")
            for k in range(NK):
                nc.tensor.matmul(psk, lhsT=w_k[:, k], rhs=xcs[k // CW][:, k % CW],
                                 start=(k == 0), stop=(k == NK - 1))
            rope_from_psum(psk, kt[:, s0:s0 + QC], s0)

            psv = ps_proj.tile([DH, QC], dt.float32, tag="acc", name="psv")
            for k in range(NK):
                nc.tensor.matmul(psv, lhsT=w_v[:, k], rhs=xcs[k // CW][:, k % CW],
                                 start=(k == 0), stop=(k == NK - 1))
            vtmp = tpool.tile([DH, QC], dt.bfloat16, tag="vtmp", name="vtmp")
            nc.scalar.copy(vtmp, psv)
            for i in range(QC // 128):
                pvt = ps_sc.tile([128, 128], dt.bfloat16, tag="sc", name="pvt")
                nc.tensor.transpose(pvt, vtmp[:, i * 128:(i + 1) * 128], c_id)
                nc.vector.tensor_copy(vn[:, sq * 4 + i], pvt)

        def emit_q(sq, h, xcs):
            """Q projection + RoPE for head h, quarter sq.

            The x-chunk read order is rotated per head so that the last Q
            stream (h3) reads chunk 0 first -> next quarter's chunk-0
            prefetch DMA (WAR on these reads) can start that much sooner,
            staggered in the same order the next quarter consumes them."""
            s0 = sq * QC
            psq = ps_proj.tile([DH, QC], dt.float32, tag="acc", name="psq")
            korder = [CW * ((2 * (h + 1) + g) % NXC) + kk
                      for g in range(NXC) for kk in range(CW)]
            for i, k in enumerate(korder):
                nc.tensor.matmul(psq, lhsT=w_q[h][:, k],
                                 rhs=xcs[k // CW][:, k % CW],
                                 start=(i == 0), stop=(i == NK - 1))
            rope_from_psum(psq, qt[h][:, s0:s0 + QC], s0)

        def emit_attn_head(sq, h):
            """Causal attention for head h over quarter sq's queries.

            Scores are [key-tile, q] transposed; diagonal key-tiles are
            trimmed to the columns that aren't fully masked, and the
            128-wide triangle on the diagonal gets the 0/1 mask."""
            s0 = sq * QC
            njt = 4 * (sq + 1)
            sacc = spool.tile([128, QC], dt.bfloat16, tag="sacc", name="sacc")
            po = ps_po.tile([DH, QC], dt.float32, tag="po", name="po")
            for jt in range(njt):
                r = jt - (njt - 4)          # >=0 -> diagonal tile index
                c0 = 128 * r if r > 0 else 0
                psc = ps_sc.tile([128, QC], dt.float32, tag="sc", name="psc")
                nc.tensor.matmul(psc[:, c0:QC], lhsT=kt[:, jt * KT:(jt + 1) * KT],
                                 rhs=qt[h][:, s0 + c0:s0 + QC],
                                 start=True, stop=True)
                e = epool.tile([128, QC], dt.bfloat16, tag="e", name="e")
                nc.scalar.activation(out=e[:, c0:QC], in_=psc[:, c0:QC],
                                     func=mybir.ActivationFunctionType.Exp,
                                     bias=c_bias, scale=SCALE)
                if r >= 0:
                    nc.vector.tensor_mul(e[:, c0:c0 + 128], e[:, c0:c0 + 128],
                                         c_tri)
                if jt == 0:
                    nc.vector.tensor_copy(sacc, e[:, 0:QC])
                else:
                    nc.vector.tensor_add(sacc[:, c0:QC], sacc[:, c0:QC],
                                         e[:, c0:QC])
                nc.tensor.matmul(po[:, c0:QC], lhsT=vn[:, jt], rhs=e[:, c0:QC],
                                 start=(jt == 0), stop=(jt == njt - 1))
            # normalization: ctx = O * (1/colsum(E)). colsum via a ones
            # matmul, fast approximate reciprocal on DVE (single custom op,
            # ~51 ULP), partition-broadcast on the otherwise-idle gpsimd,
            # DVE scale. Nothing expensive on any engine, no PE wait.
            pcs = ps_out.tile([1, QC], dt.float32, tag="pso", name="pcs")
            nc.tensor.matmul(pcs, lhsT=c_oc, rhs=sacc, start=True, stop=True)
            rec = npool.tile([1, QC], dt.float32, tag="rec", name="rec")
            nc.vector.reciprocal_approx_fast(out=rec, in_=pcs)
            rcs = npool.tile([128, QC], dt.float32, tag="nrm", name="rcs")
            nc.gpsimd.partition_broadcast(rcs, rec)
            nc.vector.tensor_mul(ctxT[h][:, s0:s0 + QC], po, rcs)

        def emit_outproj_block(st, alt_pool=None):
            """out[st*128:(st+1)*128, :] = sum_h ctxT[h][:, st-block].T @ Wo[h]."""
            for quad in range(4):
                ob = obp.tile([128, DIN // 4], dt.bfloat16, tag="ob", name="ob")
                for j in range(2):
                    oc = quad * 2 + j
                    if alt_pool is not None and oc % 2:
                        pso = alt_pool.tile([128, 512], dt.float32, tag="acc",
                                            name="pso_alt")
                    else:
                        pso = ps_out.tile([128, 512], dt.float32, tag="pso",
                                          name="pso")
                    for h in range(HPC):
                        nc.tensor.matmul(pso,
                                         lhsT=ctxT[h][:, st * 128:(st + 1) * 128],
                                         rhs=w_o[:, h, oc * 512:(oc + 1) * 512],
                                         start=(h == 0), stop=(h == HPC - 1))
                    dst = ob[:, j * 512:(j + 1) * 512]
                    if oc % 2 == 0:
                        nc.scalar.copy(dst, pso)
                    else:
                        nc.vector.tensor_copy(dst, pso)
                nc.sync.dma_start(
                    out=out[st * 128:(st + 1) * 128,
                            quad * (DIN // 4):(quad + 1) * (DIN // 4)],
                    in_=ob)

        # ---- main pipeline ----
        # Per quarter: K/V projections, then per head [Q_h, attention_h,
        # out-proj block of the previous quarter]. The out-proj matmuls are
        # independent PE filler while attention waits on exp results; the
        # x chunks for the next quarter prefetch once the last Q reads them.
        def prefetch_x(sq1, gs):
            tiles = []
            for g in gs:
                xc = xp.tile([128, NK // NXC, QC], dt.bfloat16,
                             tag=f"xc{g}", name=f"xc{g}_{sq1}")
                nc.sync.dma_start(out=xc, in_=x[:, sq1, g * CW:(g + 1) * CW])
                tiles.append(xc)
            return tiles

        xcs = xcs0
        for sq in range(NQ):
            emit_kv(sq, xcs)
            for h in range(HPC):
                emit_q(sq, h, xcs)
                if h == HPC - 1 and sq + 1 < NQ:
                    # stagger the prefetch issue so the first chunks get
                    # full DMA bandwidth and land before the next K-proj
                    nxt = prefetch_x(sq + 1, range(0, 3))
                emit_attn_head(sq, h)
                # out-proj blocks of the previous quarter fill PE bubbles at
                # heads 0-2; block 4*sq at the quarter boundary (right after
                # head 3) covers the gap while the next quarter's x arrives.
                if sq >= 1 and h < HPC - 1:
                    emit_outproj_block(4 * (sq - 1) + h + 1)
            if sq + 1 < NQ:
                nxt += prefetch_x(sq + 1, range(3, 6))
            emit_outproj_block(4 * sq)
            if sq + 1 < NQ:
                nxt += prefetch_x(sq + 1, range(6, NXC))
                xcs = nxt
        for st in range(4 * (NQ - 1) + 1, 4 * NQ):
            emit_outproj_block(st, alt_pool=ps_proj)
    nc.finalize()
    return nc


def make_in_maps(input_tensor, cos, sin, Wq, Wk, Wv, Wo):
    """Host-side sharding + layout preparation. Returns list of 8 dicts."""
    x2 = np.ascontiguousarray(input_tensor.reshape(S, DIN))
    # x_host[p, sq, k, sc] = x2[512*sq+sc, 128*k+p]
    xt = x2.T.astype(BF16)                      # [DIN, S]
    x_host = np.ascontiguousarray(
        xt.reshape(NK, 128, NQ, QC).transpose(1, 2, 0, 3))

    cosT = np.ascontiguousarray(cos.T.astype(np.float32))
    sinm = np.ascontiguousarray(sin.T.astype(np.float32))
    sinm = sinm.copy()
    sinm[0:64, :] *= -1.0

    p_idx = np.arange(128)[:, None]
    c_idx = np.arange(128)[None, :]
    tri = (p_idx <= c_idx).astype(BF16)

    ident = np.eye(128, dtype=BF16)
    ones_col = np.ones((128, 1), dtype=BF16)

    common = dict(x=x_host, cosT=cosT, sinm=sinm, tri=tri, ident=ident,
                  ones_col=ones_col)

    in_maps = []
    for c in range(NCORES):
        wq_s = Wq[:, c * DPC:(c + 1) * DPC].astype(BF16)
        wq_host = np.ascontiguousarray(
            wq_s.reshape(NK, 128, HPC, DH).transpose(1, 0, 2, 3))
        wk_s = Wk[:, c * DH:(c + 1) * DH].astype(BF16)
        wk_host = np.ascontiguousarray(wk_s.reshape(NK, 128, DH).transpose(1, 0, 2))
        wv_s = Wv[:, c * DH:(c + 1) * DH].astype(BF16)
        wv_host = np.ascontiguousarray(wv_s.reshape(NK, 128, DH).transpose(1, 0, 2))
        wo_s = Wo[c * DPC:(c + 1) * DPC, :].astype(BF16)
        wo_host = np.ascontiguousarray(wo_s.reshape(HPC, 128, DIN).transpose(1, 0, 2))
        in_maps.append(dict(common, wq=wq_host, wk=wk_host, wv=wv_host, wo=wo_host))
    return in_maps


def _numpy_fallback(input_tensor, attention_mask, cos, sin, Wq, Wk, Wv, Wo):
    x = input_tensor.astype(np.float32)
    b, s, _ = x.shape
    q = (x @ Wq).reshape(b, s, H, DH).transpose(0, 2, 1, 3)
    k = (x @ Wk).reshape(b, s, KV, DH).transpose(0, 2, 1, 3)
    v = (x @ Wv).reshape(b, s, KV, DH).transpose(0, 2, 1, 3)

    def rope(t):
        t1, t2 = t[..., :64], t[..., 64:]
        rot = np.concatenate([-t2, t1], axis=-1)
        return t * cos[None, None] + rot * sin[None, None]

    q, k = rope(q), rope(k)
    k = np.repeat(k, G, axis=1)
    v = np.repeat(v, G, axis=1)
    sc = np.einsum('bhqd,bhkd->bhqk', q, k)
    sc = np.where(attention_mask, -np.inf, sc) / np.float32(np.sqrt(DH))
    sc = sc - sc.max(axis=-1, keepdims=True)
    w = np.exp(sc)
    w = w / w.sum(axis=-1, keepdims=True)
    ctx = np.einsum('bhqk,bhkd->bhqd', w, v)
    ctx = ctx.transpose(0, 2, 1, 3).reshape(b, s, H * DH)
    return (ctx @ Wo).astype(np.float32)


_NC_CACHE = {}


def kernel(input_tensor, attention_mask, cos, sin, Wq, Wk, Wv, Wo):
    mask = np.asarray(attention_mask).reshape(S, S)
    causal = np.array_equal(mask, np.triu(np.ones((S, S), bool), k=1))
    if not causal:
        return _numpy_fallback(np.asarray(input_tensor), np.asarray(attention_mask),
                               np.asarray(cos), np.asarray(sin),
                               np.asarray(Wq), np.asarray(Wk),
                               np.asarray(Wv), np.asarray(Wo))

    if "nc" not in _NC_CACHE:
        _NC_CACHE["nc"] = build_nc()
    nc = _NC_CACHE["nc"]

    in_maps = make_in_maps(np.asarray(input_tensor), np.asarray(cos),
                           np.asarray(sin), np.asarray(Wq), np.asarray(Wk),
                           np.asarray(Wv), np.asarray(Wo))
    res = run_bass_kernel_spmd(nc, in_maps, core_ids=list(range(NCORES)))
    acc = np.zeros((S, DIN), np.float32)
    for r in res.results:
        acc += np.asarray(r["out"], dtype=np.float32)
    return acc.reshape(1, S, DIN)
